# revision 41
# baseline (speedup 1.0000x reference)
"""Trainium2 Bass kernel for nn_AttentionLayer_66949950210666.

Cross-attention layer: q from decoder_hs, k/v from encoder_hs,
16 heads, D=1024, S=2048, B=2, fp32.

Sharding (8 cores): core c = (b, r) with b = c // 4, r = c % 4.
Each core handles batch b and heads [4r, 4r+4) (o-dims [256r, 256r+256)).
Device-side, everything lives in a "transposed world":
  QT[o, s], KT[o, s]  (o on partitions)  and V[s, o] (s on partitions),
so the attention works without any on-chip transposes.

Scores use ROW-TILED PACKED matmuls: each head's K/Q live on a 64-
partition half (head hh of a pair at partitions [64hh, 64hh+64)), so the
two heads' scores matmuls are K=64 matmuls on DISTINCT PE row-groups
(tile_position (0,0) / (64,0)) and execute CONCURRENTLY — one N=512 slot
for both heads instead of two zero-padded K=128 slots.

  ST[k, q]   = KT2[pair][64hh:64hh+64]^T-contract-d  (row-tiled pair)
  PT[k, q]   = exp(ST / 8)                       (ScalarE, no max-subtract;
                                                  |S| ~ N(0,1), fp32-safe)
  YuT[d, q]  = sum_k Vaug[k, d|ones] PT[k, q]    (K=128 matmul; the extra
                                                  "ones" column makes row 64
                                                  the softmax denominator)
  YT = YuT * recip(denom)  broadcast across partitions via a tiny K=128
  row-selector matmul + reciprocal_approx_fast.

The kernel is one continuous stream: 128 exp ACTIVATEs of [128,1024]
(~142us on ScalarE) paced against the PE.  All projection matmuls are
injected into the PE stream between attention iterations against their
data deadlines.  Inputs load on TWO DMA queues (sync: xe stream;
gpsimd: weights + xd) so the q-side never waits behind the k-side.

Q-columns are HOST-PERMUTED: device column j = 512*qt + i holds logical
s = 4*i + qt, so each qt's 512 columns spread evenly over all 8
output-shard cores (64 each).  This lets the output AllToAll run as 8
small per-(pair,qt) collectives pipelined behind compute; only the last
chunk (~10us) remains on the tail.  The final projection runs in two
qt-halves: half 0 executes during the last AllToAll chunk, half 1
(~8us) after it, accumulating both pairs directly in PSUM.
"""

import sys

sys.path.insert(0, "/opt/trn_rl_repo")

import ml_dtypes
import numpy as np

import bass_rust as _bass_rust

import concourse.bass as bass
import concourse.mybir as mybir
import concourse.tile as tile
from concourse import bacc
from concourse.bass_utils import run_bass_kernel_spmd

F32 = mybir.dt.float32
BF16 = mybir.dt.bfloat16
F8 = mybir.dt.float8e4

# The greedy ACT-table chooser could ping-pong between table sets; hide
# Exp/Ln from the single-function sets so every activation resolves to
# natural_log_exp_and_others.  Only the membership sets are changed — dict
# order/length (the act_func_set_id space) is untouched.
import concourse.hw_specs as _hw_specs
from concourse import bacc as _bacc_mod

_orig_get_tables = _hw_specs.get_activation_tables


def _patched_get_tables(arch):
    t = {k: set(v) for k, v in _orig_get_tables(arch).items()}
    if "natural_log_exp_and_others" in t:
        for name, fns in t.items():
            if name != "natural_log_exp_and_others":
                fns.discard(mybir.ActivationFunctionType.Exp)
                fns.discard(mybir.ActivationFunctionType.Ln)
    return t


_bacc_mod.get_activation_tables = _patched_get_tables

B, S, D, H, HD = 2, 2048, 1024, 16, 64
NCORES = 8
OL = 256          # local output dims (4 heads x 64)
SB = S // 8       # 256: s-slice per core after the 8-way AllToAll
NST = S // 512    # 4 s-tiles of 512
NDC = D // 128    # 8 contraction chunks
NKT = S // 128    # 16 k-tiles
SCALE = 0.125     # 1/sqrt(HD)


def build_nc():
    nc = bacc.Bacc(None, num_devices=NCORES, target_bir_lowering=False)

    # All inputs are HOST-PRE-ARRANGED into the exact SBUF layout
    # ([128 partitions, free]) so every input DMA is a contiguous
    # per-partition block — strided descriptors capped DMA throughput at
    # ~50-100 GB/s per queue and starved the prologue.
    xdT = nc.declare_dram_parameter("xdT", [NST, 128, 4096], BF16, isOutput=False)
    xeT = nc.declare_dram_parameter("xeT", [NST, 128, 4096], BF16, isOutput=False)
    wqT = nc.declare_dram_parameter("wqT", [128, NDC * OL], BF16, isOutput=False)
    wkT = nc.declare_dram_parameter("wkT", [128, NDC * OL], BF16, isOutput=False)
    wvT = nc.declare_dram_parameter("wvT", [128, NDC * OL], BF16, isOutput=False)
    wpT = nc.declare_dram_parameter("wpT", [128, NDC * D], BF16, isOutput=False)
    # packed biases: rows 0-1 bq, 2-3 bk, 4-11 cb (= Wp@bv + bp)
    bAll = nc.declare_dram_parameter("bAll", [128, 12], F32, isOutput=False)
    ztO = nc.declare_dram_parameter("zT", [2, D, SB], BF16, isOutput=True)

    with tile.TileContext(nc) as tc:
        with (
            tc.tile_pool(name="const", bufs=1) as const,
            tc.tile_pool(name="big", bufs=1) as big,
            tc.tile_pool(name="xp", bufs=1) as xp,
            tc.tile_pool(name="dram", bufs=1, space="DRAM") as dram,
        ):
            # ---- constants / weights resident in SBUF ----
            wq_s = const.tile([128, NDC, OL], BF16)
            wk_s = const.tile([128, NDC, OL], BF16)
            wv_s = const.tile([128, NDC, OL], BF16)
            wp_s = const.tile([128, NDC, D], BF16)
            ball_s = const.tile([128, 12], F32)
            bq_s = ball_s[:, 0:2]
            bk_s = ball_s[:, 2:4]
            cb_s = ball_s[:, 4:12]
            # zeroed scratch for PE warm-up matmuls (no DMA dependency)
            wscr = const.tile([128, 512], BF16)
            nc.vector.memset(wscr[:], 0.0)
            # row-64 selector: out[m, q] = rhs[64, q] via a K=128 matmul
            # (row 64 ones, all other rows zero); db is a persistent,
            # pre-zeroed staging row so the matmul never reads
            # uninitialized SBUF on its zero rows.
            ones_sb = const.tile([128, 128], BF16)
            nc.vector.memset(ones_sb[:], 0.0)
            nc.vector.memset(ones_sb[64:65, :], 1.0)
            db_s = [const.tile([128, 512], BF16, name=f"db{i}") for i in range(2)]
            for i in range(2):
                nc.vector.memset(db_s[i][:], 0.0)

            # persistent activations: head hh of a pair on partitions
            # [64hh, 64hh+64) for both QT and KT2 (enables row-tiled
            # packed scores matmuls, K=64 per head, concurrent).
            QT = [big.tile([128, S], BF16, tag=f"QT{i}", name=f"QT{i}") for i in range(2)]
            KT2 = [big.tile([128, S], BF16, tag=f"KT2{i}", name=f"KT2{i}")
                   for i in range(2)]
            # V augmented with a ones column per head: [k-part, kt, h, 65]
            vaug = big.tile([128, NKT, 4, 65], BF16, tag="vaug")
            nc.vector.memset(vaug[:, :, :, 64:65], 1.0)

            # input staging: one tile + one DMA per 512-s-block.
            # Element [p, dch, two, s] holds d-index dch*256 + two*128 + p,
            # so contraction chunk dc lives at [:, dc // 2, dc % 2, :].
            xe_t = [xp.tile([128, 4, 2, 512], BF16, tag="xe", name=f"xe{st}",
                            bufs=4) for st in range(NST)]
            xd_t = [xp.tile([128, 4, 2, 512], BF16, tag="xd", name=f"xd{qt}",
                            bufs=4) for qt in range(NST)]

            # warm up the CC ring: a tiny zero-filled AllToAll triggered
            # first absorbs the ~11.5us first-collective setup during the
            # DMA prologue (triggers do not block the engine queue).
            ccw_in = dram.tile([8, 16, 64], BF16, name="ccw_in")
            ccw_out = dram.tile([8, 16, 64], BF16, name="ccw_out")
            nc.gpsimd.dma_start(
                ccw_in[:].rearrange("a b q -> (a b) q"), wscr[0:128, 0:64])
            nc.gpsimd.collective_compute(
                "AllToAll", mybir.AluOpType.bypass,
                replica_groups=[list(range(NCORES))],
                ins=[ccw_in.opt()], outs=[ccw_out.opt()])

            # ---- three-queue input load (all contiguous transfers) ----
            # sync queue: the k/v-side stream (xe st0..3).
            # gpsimd queue: weights + biases + xd1-3 + wp.
            # scalar queue: xd0 (exp(0) depends on it anyway).
            for st in range(NST):
                nc.sync.dma_start(
                    xe_t[st][:].rearrange("p a b s -> p (a b s)"),
                    xeT[st])
            nc.gpsimd.dma_start(
                wk_s[:].rearrange("p a b -> p (a b)"), wkT[:])
            nc.gpsimd.dma_start(ball_s[:], bAll[:])
            nc.gpsimd.dma_start(
                wq_s[:].rearrange("p a b -> p (a b)"), wqT[:])
            nc.scalar.dma_start(
                xd_t[0][:].rearrange("p a b s -> p (a b s)"), xdT[0])
            nc.gpsimd.dma_start(
                wv_s[:].rearrange("p a b -> p (a b)"), wvT[:])
            for qt in range(1, NST):
                nc.gpsimd.dma_start(
                    xd_t[qt][:].rearrange("p a b s -> p (a b s)"), xdT[qt])
            nc.gpsimd.dma_start(
                wp_s[:].rearrange("p a b -> p (a b)"), wpT[:])

            # AllToAll chunks: device q-columns are host-permuted so each
            # qt's 512 columns = 8 dest-cores x 64.  Pair 0 exchanges in
            # ONE 512KB collective (it finishes mid-stream; big ops have
            # better bandwidth); pair 1 in four per-qt 128KB collectives
            # so the chunk produced at stream end is as small as possible
            # (the CC stream is serial and each op has a ~10us floor).
            ydramC0 = dram.tile([8, 128, 256], BF16, name="ydram0")
            ygathC0 = dram.tile([8, 128, 256], BF16, name="ygath0")
            ydramC1 = [dram.tile([8, 128, 64], BF16, name=f"ydram1_{q}")
                       for q in range(NST)]
            ygathC1 = [dram.tile([8, 128, 64], BF16, name=f"ygath1_{q}")
                       for q in range(NST)]
            # gathered Y^T chunks: [j, bb, g, qt(-in-half), q]
            ytg0 = const.tile([128, 2, 4, 4, 64], BF16, name="ytg0")
            ytg1 = [const.tile([128, 2, 4, 2, 64], BF16, name=f"ytg1_{h}")
                    for h in range(2)]

            with (
                tc.tile_pool(name="stp", bufs=2, space="PSUM") as stp,
                tc.tile_pool(name="yup", bufs=2, space="PSUM") as yup,
                tc.tile_pool(name="aux", bufs=2, space="PSUM") as auxp,
                tc.tile_pool(name="pt", bufs=6) as ptp,
                tc.tile_pool(name="ep", bufs=6) as ep,
            ):
                # ---- injected projection groups (each uses one aux slot) ----
                def emit_k(st, oc):
                    ssl = slice(st * 512, (st + 1) * 512)
                    kps = auxp.tile([128, 512], F32, tag="aux", name="kps")
                    for dc in range(NDC):
                        nc.tensor.matmul(
                            kps[:], wk_s[:, dc, oc * 128:(oc + 1) * 128],
                            xe_t[st][:, dc // 2, dc % 2, :],
                            start=(dc == 0), stop=(dc == NDC - 1))
                    nc.vector.tensor_scalar_add(
                        KT2[oc][:, ssl], kps[:], bk_s[:, oc:oc + 1])

                def emit_v(st, half):
                    # two s-subblocks (kt = 4*st + 2*half + {0,1}) share one
                    # aux slot; one DVE copy moves both into vaug
                    vps = auxp.tile([128, 2, 256], F32, tag="aux", name="vps")
                    for uu in range(2):
                        u = 2 * half + uu
                        for dc in range(NDC):
                            nc.tensor.matmul(
                                vps[:, uu, :],
                                xe_t[st][:, dc // 2, dc % 2,
                                         u * 128:(u + 1) * 128],
                                wv_s[:, dc, :],
                                start=(dc == 0), stop=(dc == NDC - 1))
                    kt0 = 4 * st + 2 * half
                    nc.vector.tensor_copy(
                        vaug[:, kt0:kt0 + 2, :, 0:64],
                        vps[:].rearrange("p u (h d) -> p u h d", h=4))

                def emit_q(qt, oc):
                    qsl = slice(qt * 512, (qt + 1) * 512)
                    qps = auxp.tile([128, 512], F32, tag="aux", name="qps")
                    for dc in range(NDC):
                        nc.tensor.matmul(
                            qps[:], wq_s[:, dc, oc * 128:(oc + 1) * 128],
                            xd_t[qt][:, dc // 2, dc % 2, :],
                            start=(dc == 0), stop=(dc == NDC - 1))
                    nc.vector.tensor_scalar_add(
                        QT[oc][:, qsl], qps[:], bq_s[:, oc:oc + 1])

                def a2a(src, dst):
                    nc.gpsimd.collective_compute(
                        "AllToAll", mybir.AluOpType.bypass,
                        replica_groups=[list(range(NCORES))],
                        ins=[src.opt()], outs=[dst.opt()])

                def gath0():
                    nc.gpsimd.dma_start(
                        ytg0[:],
                        ygathC0.rearrange(
                            "(bb g) j (t q) -> j bb g t q", bb=2, t=4))

                def gath1(qt):
                    nc.gpsimd.dma_start(
                        ytg1[qt // 2][:, :, :, qt % 2, :],
                        ygathC1[qt].rearrange("(bb g) j q -> j bb g q", bb=2))

                def finish_qt(pair, qt, yufs, anchor):
                    # deferred normalize+store; the raw denominator row is
                    # broadcast across partitions via a K=128 selector
                    # matmul (pinned behind `anchor` so the scheduler
                    # cannot hoist it into a head-of-line block), then the
                    # fast approximate reciprocal runs at base partition 0.
                    for hh in range(2):
                        nc.vector.tensor_copy(
                            db_s[hh][64:65, :], yufs[hh][64:65, :])
                    rpss = []
                    for hh in range(2):
                        rps = auxp.tile([128, 512], F32, tag="aux", name="rps")
                        rmm = nc.tensor.matmul(
                            rps[:], ones_sb[:, :], db_s[hh][:, :],
                            start=True, stop=True)
                        _bass_rust.add_dep_helper(
                            rmm.ins, anchor.ins, sync=False,
                            reason="pin R-matmul after current attention MMs")
                        rpss.append(rps)
                    ysts = []
                    for hh in range(2):
                        rrec = ep.tile([128, 512], F32, tag="r32", name="rrec")
                        nc.vector.reciprocal_approx_fast(
                            rrec[0:64, :], rpss[hh][0:64, :])
                        yst = ep.tile([64, 512], BF16, tag="yst", name="yst")
                        nc.vector.tensor_mul(
                            yst[:], yufs[hh][0:64, :], rrec[0:64, :])
                        ysts.append(yst)
                    for hh in range(2):
                        if pair == 0:
                            dst = ydramC0[:, 64 * hh:64 * (hh + 1),
                                          64 * qt:64 * (qt + 1)]
                        else:
                            dst = ydramC1[qt][:, 64 * hh:64 * (hh + 1), :]
                        nc.sync.dma_start(
                            dst.rearrange("d j q -> j d q"),
                            ysts[hh][:].rearrange("j (d q) -> j d q", d=8))
                    # Collective triggers BLOCK the gpsimd queue until the
                    # collective completes, so the queue holds ONLY
                    # triggers and gathers, interleaved so every gather's
                    # CC is (nearly) done when the queue reaches it.
                    if pair == 0 and qt == NST - 1:
                        a2a(ydramC0, ygathC0)
                    elif pair == 1:
                        a2a(ydramC1[qt], ygathC1[qt])
                        if qt == 0:
                            gath0()
                        gath1(qt)

                # ---- PE warm-up: throwaway matmuls on zeroed scratch run
                # during the DMA wait (~8.5 -> ~20us, when the first input
                # tiles land), so the HAM clock gate holds 8/8 and the
                # real prologue runs at 2.4 GHz ----
                wup = auxp.tile([128, 512], F32, tag="aux", name="wup")
                for i in range(28):
                    nc.tensor.matmul(wup[:], wscr[:, 0:128], wscr[:],
                                     start=True, stop=True)

                # ---- prologue: minimum work before the exp stream starts ----
                emit_k(0, 0)
                emit_q(0, 0)

                # injection schedule: (pair, qt, kt) -> list of thunks.
                # Deadlines: pair0/qt0 consumes KT2[0] st_j at kt=4j and
                # vaug at kt; QT[0] qt at pair0/qt start; KT2[1]/QT[1] only
                # at pair1 (iteration 64+), so their projections ride
                # pair-0's PE slack.
                inj = {}

                def at(pair, qt, kt, fn, *a):
                    inj.setdefault((pair, qt, kt), []).append((fn, a))

                at(0, 0, 0, emit_v, 0, 1)
                at(0, 0, 1, emit_k, 1, 0)
                at(0, 0, 2, emit_v, 1, 0)
                at(0, 0, 3, emit_v, 1, 1)
                at(0, 0, 5, emit_k, 2, 0)
                at(0, 0, 6, emit_v, 2, 0)
                at(0, 0, 7, emit_v, 2, 1)
                at(0, 0, 9, emit_k, 3, 0)
                at(0, 0, 10, emit_v, 3, 0)
                at(0, 0, 11, emit_v, 3, 1)
                at(0, 0, 13, emit_q, 1, 0)
                at(0, 1, 2, emit_k, 0, 1)
                at(0, 1, 6, emit_k, 1, 1)
                at(0, 1, 10, emit_q, 2, 0)
                at(0, 2, 2, emit_k, 2, 1)
                at(0, 2, 10, emit_q, 3, 0)
                at(0, 3, 2, emit_k, 3, 1)
                at(0, 3, 10, emit_q, 0, 1)
                at(1, 0, 10, emit_q, 1, 1)
                at(1, 1, 2, emit_q, 2, 1)
                at(1, 2, 2, emit_q, 3, 1)

                # ---- the attention stream ----
                # Software-pipelined emission: the scores matmuls for
                # iteration n+2 are emitted during iteration n, so they
                # execute inside exp(n)'s window and exp(n+1) is never
                # gated on a fresh scores matmul.  The two heads' scores
                # are row-tiled (K=64, tile_position (0,0)/(64,0)) and run
                # concurrently in one N=512 slot.
                def emit_scores(pair, qt, kt):
                    sps = stp.tile([128, 1024], F32, tag="st")
                    for hh in range(2):
                        psl = slice(64 * hh, 64 * (hh + 1))
                        nc.tensor.matmul(
                            sps[:, 512 * hh:512 * (hh + 1)],
                            KT2[pair][psl, kt * 128:(kt + 1) * 128],
                            QT[pair][psl, qt * 512:(qt + 1) * 512],
                            start=True, stop=True)
                    return sps

                iters = [(pair, qt, kt) for pair in range(2)
                         for qt in range(NST) for kt in range(NKT)]
                pending = None
                yu = None
                # prime the first two scores BEFORE emit_v so exp(0) starts
                # while the first V projection is still on the PE
                sps_q = [emit_scores(*iters[0]), emit_scores(*iters[1])]
                emit_v(0, 0)
                for idx, (pair, qt, kt) in enumerate(iters):
                    if kt == 0:
                        yu = [yup.tile([128, 512], F32, tag="yu",
                                       name=f"yu{hh}") for hh in range(2)]
                    sps = sps_q.pop(0)
                    pt_t = ptp.tile([128, 1024], BF16, tag="pt")
                    nc.scalar.activation(
                        pt_t[:], sps[:],
                        mybir.ActivationFunctionType.Exp, scale=SCALE)
                    if idx + 2 < len(iters):
                        sps_q.append(emit_scores(*iters[idx + 2]))
                    for hh in range(2):
                        h = 2 * pair + hh
                        last_pv = nc.tensor.matmul(
                            yu[hh][0:65, :], vaug[:, kt, h, :],
                            pt_t[:, 512 * hh:512 * (hh + 1)],
                            start=(kt == 0), stop=(kt == NKT - 1))
                    for fn, a in inj.get((pair, qt, kt), ()):
                        fn(*a)
                    if kt == 2 and pending is not None:
                        finish_qt(pair, *pending, anchor=last_pv)
                        pending = None
                    if kt == NKT - 1:
                        # evacuate PSUM immediately (DVE only, no PE ops)
                        yufs = []
                        for hh in range(2):
                            yuf = ep.tile([65, 512], F32, tag="yuf",
                                          name="yuf")
                            nc.vector.tensor_copy(yuf[:], yu[hh][0:65, :])
                            yufs.append(yuf)
                        pending = (qt, yufs)
                        if qt == NST - 1:
                            finish_qt(pair, *pending, anchor=last_pv)
                            pending = None

            # ---- output projection, two qt-halves.  Per half, PHASED:
            # all pair-0 matmuls first (their gather lands mid-stream),
            # then all pair-1 matmuls — so the PE keeps busy during the
            # last pair-1 AllToAll chunks.  zps tiles are full banks so a
            # start=True bank-clear can never touch a neighbour. ----
            with (
                tc.tile_pool(name="zps", bufs=8, space="PSUM") as zpsp,
                tc.tile_pool(name="zt", bufs=4) as ztp,
            ):
                for hf in range(2):
                    zpss = []
                    for oc in range(NDC):
                        zps = zpsp.tile([128, 512], F32, tag="z",
                                        name=f"zps{hf}_{oc}")
                        for g in range(4):
                            nc.tensor.matmul(
                                zps[:, 0:256],
                                wp_s[:, 2 * g, oc * 128:(oc + 1) * 128],
                                ytg0[:, :, g, 2 * hf:2 * hf + 2, :],
                                start=(g == 0), stop=False)
                        zpss.append(zps)
                    for oc in range(NDC):
                        for g in range(4):
                            nc.tensor.matmul(
                                zpss[oc][:, 0:256],
                                wp_s[:, 2 * g + 1, oc * 128:(oc + 1) * 128],
                                ytg1[hf][:, :, g, :, :],
                                start=False, stop=(g == 3))
                        zt_t = ztp.tile([128, 256], BF16, tag="zt", name="zt_t")
                        # split the finalize adds across DVE and ScalarE
                        # (idle after the exp stream, reads PSUM directly)
                        # so the per-oc chain doesn't serialize on DVE
                        if oc % 2 == 0:
                            nc.vector.tensor_scalar_add(
                                zt_t[:], zpss[oc][:, 0:256], cb_s[:, oc:oc + 1])
                        else:
                            nc.scalar.activation(
                                zt_t[:], zpss[oc][:, 0:256],
                                mybir.ActivationFunctionType.Identity,
                                bias=cb_s[:, oc:oc + 1])
                        nc.sync.dma_start(
                            ztO[:, oc * 128:(oc + 1) * 128,
                                128 * hf:128 * (hf + 1)]
                            .rearrange("b p q -> p b q"),
                            zt_t[:].rearrange("p (b q) -> p b q", b=2))

    nc.compile()
    return nc


# device q-column j = 512*qt + i holds logical s = 4*i + qt
_QPERM = (4 * np.arange(512)[None, :] + np.arange(4)[:, None]).reshape(-1)
# core-local output column 64*qt + il holds logical s-offset 4*il + qt
_OPERM = (4 * np.arange(64)[None, :] + np.arange(4)[:, None]).reshape(-1)


def _sbufize_x(xT):
    # [D, S] -> [NST, 128, 4096] where st-block [p, (dch, two, s)] holds
    # d = dch*256 + two*128 + p (contiguous per-partition SBUF image)
    a = xT.reshape(4, 2, 128, S)                     # [dch, two, p, S]
    a = a.transpose(2, 0, 1, 3)                      # [p, dch, two, S]
    out = np.stack([
        np.ascontiguousarray(a[:, :, :, st * 512:(st + 1) * 512]
                             .reshape(128, 4096)) for st in range(NST)])
    return out


def _sbufize_w(wT, cols):
    # [D, cols] -> [128, 8*cols]: [p, (dc, o)] with d = dc*128 + p
    return np.ascontiguousarray(
        wT.reshape(8, 128, cols).transpose(1, 0, 2).reshape(128, 8 * cols))


def make_in_maps(decoder_hs, encoder_hs, Wq, bq, Wk, bk, Wv, bv, Wp, bp):
    dh = np.ascontiguousarray(np.asarray(decoder_hs, np.float32))
    eh = np.ascontiguousarray(np.asarray(encoder_hs, np.float32))
    Wq, Wk, Wv, Wp = (np.asarray(a, np.float32) for a in (Wq, Wk, Wv, Wp))
    bq, bk, bv, bp = (np.asarray(a, np.float32) for a in (bq, bk, bv, bp))
    c = (Wp @ bv + bp).astype(np.float32)
    bf = ml_dtypes.bfloat16
    wpT = _sbufize_w(Wp.T.astype(bf), D)
    xdT = [_sbufize_x(dh[b].T[:, _QPERM].astype(bf)) for b in range(B)]
    xeT = [_sbufize_x(eh[b].T.astype(bf)) for b in range(B)]
    in_maps = []
    for core in range(NCORES):
        b, r = divmod(core, 4)
        sl = slice(OL * r, OL * (r + 1))
        ball = np.concatenate(
            [bq[sl].reshape(2, 128), bk[sl].reshape(2, 128),
             c.reshape(8, 128)], axis=0)
        in_maps.append({
            "xdT": xdT[b],
            "xeT": xeT[b],
            "wqT": _sbufize_w(Wq[sl].T.astype(bf), OL),
            "wkT": _sbufize_w(Wk[sl].T.astype(bf), OL),
            "wvT": _sbufize_w(Wv[sl].T.astype(bf), OL),
            "wpT": wpT,
            "bAll": np.ascontiguousarray(ball.T),
        })
    return in_maps


def assemble_output(results):
    out = np.empty((B, S, D), np.float32)
    for core in range(NCORES):
        zT = np.asarray(results[core]["zT"])  # [2, 1024, 256]
        for b in range(B):
            out[b, SB * core + _OPERM, :] = zT[b].T
    return out


_NC = None


def kernel(**inputs):
    global _NC
    if _NC is None:
        _NC = build_nc()
    in_maps = make_in_maps(**inputs)
    res = run_bass_kernel_spmd(_NC, in_maps, list(range(NCORES)))
    return assemble_output(res.results)


if __name__ == "__main__":
    nc = build_nc()
    print("built ok")


# revision 42
# speedup vs baseline: 1.0520x; 1.0520x over previous
"""Trainium2 Bass kernel for nn_AttentionLayer_66949950210666.

Cross-attention layer: q from decoder_hs, k/v from encoder_hs,
16 heads, D=1024, S=2048, B=2, fp32.

Sharding (8 cores): core c = (b, r) with b = c // 4, r = c % 4.
Each core handles batch b and heads [4r, 4r+4) (o-dims [256r, 256r+256)).
Device-side, everything lives in a "transposed world":
  QT[o, s], KT[o, s]  (o on partitions)  and V[s, o] (s on partitions),
so the attention works without any on-chip transposes.

Scores use ROW-TILED PACKED matmuls: each head's K/Q live on a 64-
partition half (head hh of a pair at partitions [64hh, 64hh+64)), so the
two heads' scores matmuls are K=64 matmuls on DISTINCT PE row-groups
(tile_position (0,0) / (64,0)) and execute CONCURRENTLY — one N=512 slot
for both heads instead of two zero-padded K=128 slots.

  ST[k, q]   = KT2[pair][64hh:64hh+64]^T-contract-d  (row-tiled pair)
  PT[k, q]   = exp(ST / 8)                       (ScalarE, no max-subtract;
                                                  |S| ~ N(0,1), fp32-safe)
  YuT[d, q]  = sum_k Vaug[k, d|ones] PT[k, q]    (K=128 matmul; the extra
                                                  "ones" column makes row 64
                                                  the softmax denominator)
  YT = YuT * recip(denom)  broadcast across partitions via a tiny K=128
  row-selector matmul + reciprocal_approx_fast.

The kernel is one continuous stream: 128 exp ACTIVATEs of [128,1024]
(~142us on ScalarE) paced against the PE.  All projection matmuls are
injected into the PE stream between attention iterations against their
data deadlines.  Inputs load on TWO DMA queues (sync: xe stream;
gpsimd: weights + xd) so the q-side never waits behind the k-side.

Q-columns are HOST-PERMUTED: device column j = 512*qt + i holds logical
s = 4*i + qt, so each qt's 512 columns spread evenly over all 8
output-shard cores (64 each).  This lets the output AllToAll run as 8
small per-(pair,qt) collectives pipelined behind compute; only the last
chunk (~10us) remains on the tail.  The final projection runs in two
qt-halves: half 0 executes during the last AllToAll chunk, half 1
(~8us) after it, accumulating both pairs directly in PSUM.
"""

import sys

sys.path.insert(0, "/opt/trn_rl_repo")

import ml_dtypes
import numpy as np

import bass_rust as _bass_rust

import concourse.bass as bass
import concourse.mybir as mybir
import concourse.tile as tile
from concourse import bacc
from concourse.bass_utils import run_bass_kernel_spmd

F32 = mybir.dt.float32
BF16 = mybir.dt.bfloat16
F8 = mybir.dt.float8e4

# The greedy ACT-table chooser could ping-pong between table sets; hide
# Exp/Ln from the single-function sets so every activation resolves to
# natural_log_exp_and_others.  Only the membership sets are changed — dict
# order/length (the act_func_set_id space) is untouched.
import concourse.hw_specs as _hw_specs
from concourse import bacc as _bacc_mod

_orig_get_tables = _hw_specs.get_activation_tables


def _patched_get_tables(arch):
    t = {k: set(v) for k, v in _orig_get_tables(arch).items()}
    if "natural_log_exp_and_others" in t:
        for name, fns in t.items():
            if name != "natural_log_exp_and_others":
                fns.discard(mybir.ActivationFunctionType.Exp)
                fns.discard(mybir.ActivationFunctionType.Ln)
    return t


_bacc_mod.get_activation_tables = _patched_get_tables

B, S, D, H, HD = 2, 2048, 1024, 16, 64
NCORES = 8
OL = 256          # local output dims (4 heads x 64)
SB = S // 8       # 256: s-slice per core after the 8-way AllToAll
NST = S // 512    # 4 s-tiles of 512
NDC = D // 128    # 8 contraction chunks
NKT = S // 128    # 16 k-tiles
SCALE = 0.125     # 1/sqrt(HD)


def build_nc():
    nc = bacc.Bacc(None, num_devices=NCORES, target_bir_lowering=False)

    # All inputs are HOST-PRE-ARRANGED into the exact SBUF layout
    # ([128 partitions, free]) so every input DMA is a contiguous
    # per-partition block — strided descriptors capped DMA throughput at
    # ~50-100 GB/s per queue and starved the prologue.
    xdT = nc.declare_dram_parameter("xdT", [NST, 128, 4096], BF16, isOutput=False)
    xeT = nc.declare_dram_parameter("xeT", [NST, 128, 4096], BF16, isOutput=False)
    wqT = nc.declare_dram_parameter("wqT", [128, NDC * OL], BF16, isOutput=False)
    wkT = nc.declare_dram_parameter("wkT", [128, NDC * OL], BF16, isOutput=False)
    wvT = nc.declare_dram_parameter("wvT", [128, NDC * OL], BF16, isOutput=False)
    wpT = nc.declare_dram_parameter("wpT", [128, NDC * D], BF16, isOutput=False)
    # packed biases: rows 0-1 bq, 2-3 bk, 4-11 cb (= Wp@bv + bp)
    bAll = nc.declare_dram_parameter("bAll", [128, 12], F32, isOutput=False)
    ztO = nc.declare_dram_parameter("zT", [2, D, SB], BF16, isOutput=True)

    with tile.TileContext(nc) as tc:
        with (
            tc.tile_pool(name="const", bufs=1) as const,
            tc.tile_pool(name="big", bufs=1) as big,
            tc.tile_pool(name="xp", bufs=1) as xp,
            tc.tile_pool(name="dram", bufs=1, space="DRAM") as dram,
        ):
            # ---- constants / weights resident in SBUF ----
            wq_s = const.tile([128, NDC, OL], BF16)
            wk_s = const.tile([128, NDC, OL], BF16)
            wv_s = const.tile([128, NDC, OL], BF16)
            wp_s = const.tile([128, NDC, D], BF16)
            ball_s = const.tile([128, 12], F32)
            bq_s = ball_s[:, 0:2]
            bk_s = ball_s[:, 2:4]
            cb_s = ball_s[:, 4:12]
            # zeroed scratch for PE warm-up matmuls (no DMA dependency)
            wscr = const.tile([128, 512], BF16)
            nc.vector.memset(wscr[:], 0.0)
            # row-64 selector: out[m, q] = rhs[64, q] via a K=128 matmul
            # (row 64 ones, all other rows zero); db is a persistent,
            # pre-zeroed staging row so the matmul never reads
            # uninitialized SBUF on its zero rows.
            ones_sb = const.tile([128, 128], BF16)
            nc.vector.memset(ones_sb[:], 0.0)
            nc.vector.memset(ones_sb[64:65, :], 1.0)
            db_s = [const.tile([128, 512], BF16, name=f"db{i}") for i in range(2)]
            for i in range(2):
                nc.vector.memset(db_s[i][:], 0.0)

            # persistent activations: head hh of a pair on partitions
            # [64hh, 64hh+64) for both QT and KT2 (enables row-tiled
            # packed scores matmuls, K=64 per head, concurrent).
            QT = [big.tile([128, S], BF16, tag=f"QT{i}", name=f"QT{i}") for i in range(2)]
            KT2 = [big.tile([128, S], BF16, tag=f"KT2{i}", name=f"KT2{i}")
                   for i in range(2)]
            # V augmented with a ones column per head: [k-part, kt, h, 65]
            vaug = big.tile([128, NKT, 4, 65], BF16, tag="vaug")
            nc.vector.memset(vaug[:, :, :, 64:65], 1.0)

            # input staging: one tile + one DMA per 512-s-block.
            # Element [p, dch, two, s] holds d-index dch*256 + two*128 + p,
            # so contraction chunk dc lives at [:, dc // 2, dc % 2, :].
            xe_t = [xp.tile([128, 4, 2, 512], BF16, tag="xe", name=f"xe{st}",
                            bufs=4) for st in range(NST)]
            xd_t = [xp.tile([128, 4, 2, 512], BF16, tag="xd", name=f"xd{qt}",
                            bufs=4) for qt in range(NST)]

            # warm up the CC ring: a tiny zero-filled AllToAll triggered
            # first absorbs the ~11.5us first-collective setup during the
            # DMA prologue (triggers do not block the engine queue).
            ccw_in = dram.tile([8, 16, 64], BF16, name="ccw_in")
            ccw_out = dram.tile([8, 16, 64], BF16, name="ccw_out")
            nc.gpsimd.dma_start(
                ccw_in[:].rearrange("a b q -> (a b) q"), wscr[0:128, 0:64])
            nc.gpsimd.collective_compute(
                "AllToAll", mybir.AluOpType.bypass,
                replica_groups=[list(range(NCORES))],
                ins=[ccw_in.opt()], outs=[ccw_out.opt()])

            # ---- three-queue input load (all contiguous transfers) ----
            # sync queue: the k/v-side stream (xe st0..3).
            # gpsimd queue: weights + biases + xd1-3 + wp.
            # scalar queue: xd0 (exp(0) depends on it anyway).
            for st in range(NST):
                nc.sync.dma_start(
                    xe_t[st][:].rearrange("p a b s -> p (a b s)"),
                    xeT[st])
            nc.gpsimd.dma_start(
                wk_s[:].rearrange("p a b -> p (a b)"), wkT[:])
            nc.gpsimd.dma_start(ball_s[:], bAll[:])
            nc.gpsimd.dma_start(
                wq_s[:].rearrange("p a b -> p (a b)"), wqT[:])
            nc.scalar.dma_start(
                xd_t[0][:].rearrange("p a b s -> p (a b s)"), xdT[0])
            nc.gpsimd.dma_start(
                wv_s[:].rearrange("p a b -> p (a b)"), wvT[:])
            for qt in range(1, NST):
                nc.gpsimd.dma_start(
                    xd_t[qt][:].rearrange("p a b s -> p (a b s)"), xdT[qt])
            nc.gpsimd.dma_start(
                wp_s[:].rearrange("p a b -> p (a b)"), wpT[:])

            # AllToAll chunks: device q-columns are host-permuted so each
            # qt's 512 columns = 8 dest-cores x 64.  Pair 0 exchanges in
            # ONE 512KB collective (it finishes mid-stream; big ops have
            # better bandwidth); pair 1 in four per-qt 128KB collectives
            # so the chunk produced at stream end is as small as possible
            # (the CC stream is serial and each op has a ~10us floor).
            ydramC0 = dram.tile([8, 128, 256], BF16, name="ydram0")
            ygathC0 = dram.tile([8, 128, 256], BF16, name="ygath0")
            ydramC1 = [dram.tile([8, 128, 64], BF16, name=f"ydram1_{q}")
                       for q in range(NST)]
            ygathC1 = [dram.tile([8, 128, 64], BF16, name=f"ygath1_{q}")
                       for q in range(NST)]
            # gathered Y^T chunks: [j, bb, g, qt(-in-half), q]
            ytg0 = const.tile([128, 2, 4, 4, 64], BF16, name="ytg0")
            ytg1 = [const.tile([128, 2, 4, 2, 64], BF16, name=f"ytg1_{h}")
                    for h in range(2)]

            with (
                tc.tile_pool(name="stp", bufs=2, space="PSUM") as stp,
                tc.tile_pool(name="yup", bufs=2, space="PSUM") as yup,
                tc.tile_pool(name="aux", bufs=2, space="PSUM") as auxp,
                tc.tile_pool(name="pt", bufs=6) as ptp,
                tc.tile_pool(name="ep", bufs=6) as ep,
            ):
                # ---- injected projection groups (each uses one aux slot) ----
                def emit_k(st, oc):
                    ssl = slice(st * 512, (st + 1) * 512)
                    kps = auxp.tile([128, 512], F32, tag="aux", name="kps")
                    for dc in range(NDC):
                        nc.tensor.matmul(
                            kps[:], wk_s[:, dc, oc * 128:(oc + 1) * 128],
                            xe_t[st][:, dc // 2, dc % 2, :],
                            start=(dc == 0), stop=(dc == NDC - 1))
                    nc.vector.tensor_scalar_add(
                        KT2[oc][:, ssl], kps[:], bk_s[:, oc:oc + 1])

                def emit_v(st, half):
                    # two s-subblocks (kt = 4*st + 2*half + {0,1}) share one
                    # aux slot; one DVE copy moves both into vaug
                    vps = auxp.tile([128, 2, 256], F32, tag="aux", name="vps")
                    for uu in range(2):
                        u = 2 * half + uu
                        for dc in range(NDC):
                            nc.tensor.matmul(
                                vps[:, uu, :],
                                xe_t[st][:, dc // 2, dc % 2,
                                         u * 128:(u + 1) * 128],
                                wv_s[:, dc, :],
                                start=(dc == 0), stop=(dc == NDC - 1))
                    kt0 = 4 * st + 2 * half
                    nc.vector.tensor_copy(
                        vaug[:, kt0:kt0 + 2, :, 0:64],
                        vps[:].rearrange("p u (h d) -> p u h d", h=4))

                def emit_q(qt, oc):
                    qsl = slice(qt * 512, (qt + 1) * 512)
                    qps = auxp.tile([128, 512], F32, tag="aux", name="qps")
                    for dc in range(NDC):
                        nc.tensor.matmul(
                            qps[:], wq_s[:, dc, oc * 128:(oc + 1) * 128],
                            xd_t[qt][:, dc // 2, dc % 2, :],
                            start=(dc == 0), stop=(dc == NDC - 1))
                    nc.vector.tensor_scalar_add(
                        QT[oc][:, qsl], qps[:], bq_s[:, oc:oc + 1])

                def a2a(src, dst):
                    nc.gpsimd.collective_compute(
                        "AllToAll", mybir.AluOpType.bypass,
                        replica_groups=[list(range(NCORES))],
                        ins=[src.opt()], outs=[dst.opt()])

                def gath0():
                    nc.gpsimd.dma_start(
                        ytg0[:],
                        ygathC0.rearrange(
                            "(bb g) j (t q) -> j bb g t q", bb=2, t=4))

                def gath1(qt):
                    nc.gpsimd.dma_start(
                        ytg1[qt // 2][:, :, :, qt % 2, :],
                        ygathC1[qt].rearrange("(bb g) j q -> j bb g q", bb=2))

                def finish_qt(pair, qt, yufs, anchor):
                    # deferred normalize+store; the raw denominator row is
                    # broadcast across partitions via a K=128 selector
                    # matmul (pinned behind `anchor` so the scheduler
                    # cannot hoist it into a head-of-line block), then the
                    # fast approximate reciprocal runs at base partition 0.
                    for hh in range(2):
                        nc.vector.tensor_copy(
                            db_s[hh][64:65, :], yufs[hh][64:65, :])
                    rpss = []
                    for hh in range(2):
                        rps = auxp.tile([128, 512], F32, tag="aux", name="rps")
                        rmm = nc.tensor.matmul(
                            rps[:], ones_sb[:, :], db_s[hh][:, :],
                            start=True, stop=True)
                        _bass_rust.add_dep_helper(
                            rmm.ins, anchor.ins, sync=False,
                            reason="pin R-matmul after current attention MMs")
                        rpss.append(rps)
                    ysts = []
                    for hh in range(2):
                        rrec = ep.tile([128, 512], F32, tag="r32", name="rrec")
                        nc.vector.reciprocal_approx_fast(
                            rrec[0:64, :], rpss[hh][0:64, :])
                        yst = ep.tile([64, 512], BF16, tag="yst", name="yst")
                        nc.vector.tensor_mul(
                            yst[:], yufs[hh][0:64, :], rrec[0:64, :])
                        ysts.append(yst)
                    for hh in range(2):
                        if pair == 0:
                            dst = ydramC0[:, 64 * hh:64 * (hh + 1),
                                          64 * qt:64 * (qt + 1)]
                        else:
                            dst = ydramC1[qt][:, 64 * hh:64 * (hh + 1), :]
                        nc.sync.dma_start(
                            dst.rearrange("d j q -> j d q"),
                            ysts[hh][:].rearrange("j (d q) -> j d q", d=8))
                    # Collective triggers BLOCK the gpsimd queue until the
                    # collective completes, so the queue holds ONLY
                    # triggers and gathers, interleaved so every gather's
                    # CC is (nearly) done when the queue reaches it.
                    if pair == 0 and qt == NST - 1:
                        a2a(ydramC0, ygathC0)
                    elif pair == 1:
                        a2a(ydramC1[qt], ygathC1[qt])
                        if qt == 0:
                            gath0()
                        gath1(qt)

                # ---- PE warm-up: throwaway matmuls on zeroed scratch run
                # during the DMA wait (~8.5 -> ~20us, when the first input
                # tiles land), so the HAM clock gate holds 8/8 and the
                # real prologue runs at 2.4 GHz ----
                wup = auxp.tile([128, 512], F32, tag="aux", name="wup")
                for i in range(28):
                    nc.tensor.matmul(wup[:], wscr[:, 0:128], wscr[:],
                                     start=True, stop=True)

                # ---- prologue: minimum work before the exp stream starts ----
                emit_k(0, 0)
                emit_q(0, 0)

                # injection schedule: (pair, qt, kt) -> list of thunks.
                # Deadlines: pair0/qt0 consumes KT2[0] st_j at kt=4j and
                # vaug at kt; QT[0] qt at pair0/qt start; KT2[1]/QT[1] only
                # at pair1 (iteration 64+), so their projections ride
                # pair-0's PE slack.
                inj = {}

                def at(pair, qt, kt, fn, *a):
                    inj.setdefault((pair, qt, kt), []).append((fn, a))

                at(0, 0, 0, emit_v, 0, 1)
                at(0, 0, 1, emit_k, 1, 0)
                at(0, 0, 2, emit_v, 1, 0)
                at(0, 0, 3, emit_v, 1, 1)
                at(0, 0, 5, emit_k, 2, 0)
                at(0, 0, 6, emit_v, 2, 0)
                at(0, 0, 7, emit_v, 2, 1)
                at(0, 0, 9, emit_k, 3, 0)
                at(0, 0, 10, emit_v, 3, 0)
                at(0, 0, 11, emit_v, 3, 1)
                at(0, 0, 13, emit_q, 1, 0)
                at(0, 1, 2, emit_k, 0, 1)
                at(0, 1, 6, emit_k, 1, 1)
                at(0, 1, 10, emit_q, 2, 0)
                at(0, 2, 2, emit_k, 2, 1)
                at(0, 2, 10, emit_q, 3, 0)
                at(0, 3, 2, emit_k, 3, 1)
                at(0, 3, 10, emit_q, 0, 1)
                at(1, 0, 10, emit_q, 1, 1)
                at(1, 1, 2, emit_q, 2, 1)
                at(1, 2, 2, emit_q, 3, 1)

                # ---- the attention stream ----
                # Software-pipelined emission: the scores matmuls for
                # iteration n+2 are emitted during iteration n, so they
                # execute inside exp(n)'s window and exp(n+1) is never
                # gated on a fresh scores matmul.  The two heads' scores
                # are row-tiled (K=64, tile_position (0,0)/(64,0)) and run
                # concurrently in one N=512 slot.
                def emit_scores(pair, qt, kt):
                    sps = stp.tile([128, 1024], F32, tag="st")
                    for hh in range(2):
                        psl = slice(64 * hh, 64 * (hh + 1))
                        nc.tensor.matmul(
                            sps[:, 512 * hh:512 * (hh + 1)],
                            KT2[pair][psl, kt * 128:(kt + 1) * 128],
                            QT[pair][psl, qt * 512:(qt + 1) * 512],
                            start=True, stop=True)
                    return sps

                iters = [(pair, qt, kt) for pair in range(2)
                         for qt in range(NST) for kt in range(NKT)]
                pending = None
                yu = None
                # prime the first two scores BEFORE emit_v so exp(0) starts
                # while the first V projection is still on the PE
                sps_q = [emit_scores(*iters[0]), emit_scores(*iters[1])]
                emit_v(0, 0)
                for idx, (pair, qt, kt) in enumerate(iters):
                    if kt == 0:
                        yu = [yup.tile([128, 512], F32, tag="yu",
                                       name=f"yu{hh}") for hh in range(2)]
                    sps = sps_q.pop(0)
                    pt_t = ptp.tile([128, 1024], BF16, tag="pt")
                    nc.scalar.activation(
                        pt_t[:], sps[:],
                        mybir.ActivationFunctionType.Exp, scale=SCALE)
                    if idx + 2 < len(iters):
                        sps_q.append(emit_scores(*iters[idx + 2]))
                    for hh in range(2):
                        h = 2 * pair + hh
                        last_pv = nc.tensor.matmul(
                            yu[hh][0:65, :], vaug[:, kt, h, :],
                            pt_t[:, 512 * hh:512 * (hh + 1)],
                            start=(kt == 0), stop=(kt == NKT - 1))
                    for fn, a in inj.get((pair, qt, kt), ()):
                        fn(*a)
                    if kt == 2 and pending is not None:
                        finish_qt(pair, *pending, anchor=last_pv)
                        pending = None
                    if kt == NKT - 1:
                        # evacuate PSUM immediately (DVE only, no PE ops)
                        yufs = []
                        for hh in range(2):
                            yuf = ep.tile([65, 512], F32, tag="yuf",
                                          name="yuf")
                            nc.vector.tensor_copy(yuf[:], yu[hh][0:65, :])
                            yufs.append(yuf)
                        pending = (qt, yufs)
                        if qt == NST - 1:
                            finish_qt(pair, *pending, anchor=last_pv)
                            pending = None

            # ---- output projection, two qt-halves.  Per half, PHASED:
            # all pair-0 matmuls first (their gather lands mid-stream),
            # then all pair-1 matmuls — so the PE keeps busy during the
            # last pair-1 AllToAll chunks.  zps tiles are full banks so a
            # start=True bank-clear can never touch a neighbour. ----
            with (
                tc.tile_pool(name="zps", bufs=8, space="PSUM") as zpsp,
                tc.tile_pool(name="zt", bufs=4) as ztp,
            ):
                for hf in range(2):
                    zpss = []
                    for oc in range(NDC):
                        zps = zpsp.tile([128, 512], F32, tag="z",
                                        name=f"zps{hf}_{oc}")
                        for g in range(4):
                            nc.tensor.matmul(
                                zps[:, 0:256],
                                wp_s[:, 2 * g, oc * 128:(oc + 1) * 128],
                                ytg0[:, :, g, 2 * hf:2 * hf + 2, :],
                                start=(g == 0), stop=False)
                        zpss.append(zps)
                    for oc in range(NDC):
                        for g in range(4):
                            nc.tensor.matmul(
                                zpss[oc][:, 0:256],
                                wp_s[:, 2 * g + 1, oc * 128:(oc + 1) * 128],
                                ytg1[hf][:, :, g, :, :],
                                start=False, stop=(g == 3))
                        zt_t = ztp.tile([128, 256], BF16, tag="zt", name="zt_t")
                        nc.vector.tensor_scalar_add(
                            zt_t[:], zpss[oc][:, 0:256], cb_s[:, oc:oc + 1])
                        nc.sync.dma_start(
                            ztO[:, oc * 128:(oc + 1) * 128,
                                128 * hf:128 * (hf + 1)]
                            .rearrange("b p q -> p b q"),
                            zt_t[:].rearrange("p (b q) -> p b q", b=2))

    nc.compile()
    return nc


# device q-column j = 512*qt + i holds logical s = 4*i + qt
_QPERM = (4 * np.arange(512)[None, :] + np.arange(4)[:, None]).reshape(-1)
# core-local output column 64*qt + il holds logical s-offset 4*il + qt
_OPERM = (4 * np.arange(64)[None, :] + np.arange(4)[:, None]).reshape(-1)


def _sbufize_x(xT):
    # [D, S] -> [NST, 128, 4096] where st-block [p, (dch, two, s)] holds
    # d = dch*256 + two*128 + p (contiguous per-partition SBUF image)
    a = xT.reshape(4, 2, 128, S)                     # [dch, two, p, S]
    a = a.transpose(2, 0, 1, 3)                      # [p, dch, two, S]
    out = np.stack([
        np.ascontiguousarray(a[:, :, :, st * 512:(st + 1) * 512]
                             .reshape(128, 4096)) for st in range(NST)])
    return out


def _sbufize_w(wT, cols):
    # [D, cols] -> [128, 8*cols]: [p, (dc, o)] with d = dc*128 + p
    return np.ascontiguousarray(
        wT.reshape(8, 128, cols).transpose(1, 0, 2).reshape(128, 8 * cols))


def make_in_maps(decoder_hs, encoder_hs, Wq, bq, Wk, bk, Wv, bv, Wp, bp):
    dh = np.ascontiguousarray(np.asarray(decoder_hs, np.float32))
    eh = np.ascontiguousarray(np.asarray(encoder_hs, np.float32))
    Wq, Wk, Wv, Wp = (np.asarray(a, np.float32) for a in (Wq, Wk, Wv, Wp))
    bq, bk, bv, bp = (np.asarray(a, np.float32) for a in (bq, bk, bv, bp))
    c = (Wp @ bv + bp).astype(np.float32)
    bf = ml_dtypes.bfloat16
    wpT = _sbufize_w(Wp.T.astype(bf), D)
    xdT = [_sbufize_x(dh[b].T[:, _QPERM].astype(bf)) for b in range(B)]
    xeT = [_sbufize_x(eh[b].T.astype(bf)) for b in range(B)]
    in_maps = []
    for core in range(NCORES):
        b, r = divmod(core, 4)
        sl = slice(OL * r, OL * (r + 1))
        ball = np.concatenate(
            [bq[sl].reshape(2, 128), bk[sl].reshape(2, 128),
             c.reshape(8, 128)], axis=0)
        in_maps.append({
            "xdT": xdT[b],
            "xeT": xeT[b],
            "wqT": _sbufize_w(Wq[sl].T.astype(bf), OL),
            "wkT": _sbufize_w(Wk[sl].T.astype(bf), OL),
            "wvT": _sbufize_w(Wv[sl].T.astype(bf), OL),
            "wpT": wpT,
            "bAll": np.ascontiguousarray(ball.T),
        })
    return in_maps


def assemble_output(results):
    out = np.empty((B, S, D), np.float32)
    for core in range(NCORES):
        zT = np.asarray(results[core]["zT"])  # [2, 1024, 256]
        for b in range(B):
            out[b, SB * core + _OPERM, :] = zT[b].T
    return out


_NC = None


def kernel(**inputs):
    global _NC
    if _NC is None:
        _NC = build_nc()
    in_maps = make_in_maps(**inputs)
    res = run_bass_kernel_spmd(_NC, in_maps, list(range(NCORES)))
    return assemble_output(res.results)


if __name__ == "__main__":
    nc = build_nc()
    print("built ok")
